# revision 44
# baseline (speedup 1.0000x reference)
"""GQA attention kernel for 8 TRN2 NeuronCores.

Sharding: DP over batch (2) x TP over heads (4 shards): each core gets
4 Q heads + 2 KV heads of one batch. Host pre-transposes/retiles inputs,
device computes QKV proj + QK-RMSNorm + RoPE + causal attention + o_proj
partial; host sums the 4 o_proj partials per batch.

Structure (~300us, vs 318.6us baseline; PE busy ~84%):
  Phase 1 (i = 0..15 s-tiles): QKV proj into PSUM [PE]; per-subhead
    one wide Square [scalar] + reduce [vector] for the rms stats;
    Sqrt [scalar]; reciprocal + norm-mul + RoPE (all on vector -- gpsimd
    ops are 2x slower and poisoned the transpose chain); PE transposes
    into persistent qT/kT. Pair 0's QK+exp is primed mid-phase (m==2), and
    the rope(15) latency at the phase boundary is filled by emitting
    PV(0)+QK(1) between tile 15's projection and its transposes.
  Phase 2 (pairs [2,3,4,7,6,5] after the primed 0,1; the big pairs sit
    where o_proj runway exists): per pair, QK scores for all 4 heads
    first (gives the serial Exp stream a runway) [PE] -> Exp [scalar]
    -> causal diag masks [vector] -> o_proj of the PREVIOUS pair
    (ct-outer accumulation in psA, 2 half-row DMAs per s-tile overlap
    cast/writeback) covers the Exp drain -> PV with ones-column
    denominator + normalize [vector] -> attn-out transpose [PE].
  Scalar runs ONLY Square/Sqrt in phase 1 and ONLY Exp in phase 2
  (plus the one primed pair), so activation-table reloads are rare
  (baseline swapped 19x at 1.3us each).

DMA: all host arrays are laid out so every transfer moves 2-16 KB of
contiguous bytes per partition line (512 B lines ran the queues at a
third of peak). w is split into 16 per-dt stripes spread over the 3 HW
queues (sync/scalar/gpsimd) ordered by first-need time; xs0/xs1 lead
the sync queue; wo and the cos/sin bulk are deferred via trigger
placement inside the loop so the startup-critical window only carries
w + xs + cos/sin(tiles 0-1) + ident.

All matmuls run in bf16 (1 PE cycle/row); accumulation is fp32 in PSUM.
No max-subtraction in softmax: RMSNorm bounds |q.k|/sqrt(hd) <= 11.32
so exp() is fp32-safe.
"""

import numpy as np
import ml_dtypes

import concourse.bass as bass
import concourse.mybir as mybir
from concourse import bacc
from concourse.tile import TileContext
from concourse.bass_utils import run_bass_kernel_spmd

B, S, D = 2, 2048, 2048
H, KVH, HD = 16, 8, 128
NSH = 4          # TP shards per batch
HLOC = H // NSH  # 4 q heads per core
KVLOC = KVH // NSH
OC = HLOC * HD   # 512 attn-out channels per core
ST = S // 128    # 16 s-tiles
DT = D // 128    # 16 d-tiles
VW = 132         # v row width: 128 hd + 1 ones + 3 pad
SCALE = 1.0 / np.sqrt(HD)
PAIR_ORDER = [0, 1, 2, 3, 4, 7, 6, 5]

BF16 = mybir.dt.bfloat16
F32 = mybir.dt.float32
AF = mybir.ActivationFunctionType
ALU = mybir.AluOpType

_cache = {}


def build_nc():
    nc = bacc.Bacc()

    xt = nc.declare_dram_parameter("xt", [ST // 2, 128, DT * 256], BF16, isOutput=False)
    wqkv = nc.declare_dram_parameter("wqkv", [4, 128, 4 * 1024], BF16, isOutput=False)
    wot = nc.declare_dram_parameter("wot", [128, HLOC * D], BF16, isOutput=False)
    qcos = nc.declare_dram_parameter("qcos", [128, ST * HD], BF16, isOutput=False)
    qsin = nc.declare_dram_parameter("qsin", [128, ST * HD], BF16, isOutput=False)
    kcos = nc.declare_dram_parameter("kcos", [128, ST * HD], BF16, isOutput=False)
    ksin = nc.declare_dram_parameter("ksin", [128, ST * HD], BF16, isOutput=False)
    maskp = nc.declare_dram_parameter("mask", [128, 128], BF16, isOutput=False)
    identp = nc.declare_dram_parameter("ident", [128, 128], BF16, isOutput=False)
    out = nc.declare_dram_parameter("out", [S, D], BF16, isOutput=True)

    with TileContext(nc) as tc:
        with (
            tc.tile_pool(name="const", bufs=1) as constp,
            tc.tile_pool(name="xs", bufs=2) as xsp,
            tc.tile_pool(name="work", bufs=3) as workp,
            tc.tile_pool(name="small", bufs=4) as smallp,
            tc.tile_pool(name="at", bufs=2) as atp,
            tc.tile_pool(name="psA", bufs=2, space="PSUM") as psA,
            tc.tile_pool(name="psS", bufs=2, space="PSUM") as psS,
            tc.tile_pool(name="psO", bufs=2, space="PSUM") as psO,
        ):
            # ---- persistent tiles / DMA issue order (by first-need time) ----
            # w is split into 16 per-dt transfers round-robined over the 3
            # queues so the dt-loop of tile 0 starts after ~256KB instead of
            # waiting for a whole 1MB chunk. wo and the cos/sin bulk are
            # deferred into the loop, clear of the startup-critical window.
            w_sb = constp.tile([128, DT * 1024], BF16, tag="w")
            xs0 = xsp.tile([128, DT * 256], BF16, tag="xs")
            nc.sync.dma_start(out=xs0, in_=xt[0])
            xs1 = xsp.tile([128, DT * 256], BF16, tag="xs")
            nc.sync.dma_start(out=xs1, in_=xt[1])
            id_sb = constp.tile([128, 128], BF16, tag="ident")
            nc.scalar.dma_start(out=id_sb, in_=identp[:])
            cs_tiles = {}
            for nm, prm in (("qc", qcos), ("qs", qsin), ("kc", kcos), ("ks", ksin)):
                t = constp.tile([128, ST * HD], BF16, tag=nm)
                cs_tiles[nm] = t
            # queue start latencies: gpsimd ~10us, scalar ~12.5, sync ~8.8
            # (but sync first ships xs0). Stripe dts so each arrives just
            # before the dt-loop needs it; xs1 follows on sync.
            def w_dt(eng, dt):
                eng.dma_start(out=w_sb[:, dt * 1024:(dt + 1) * 1024],
                              in_=wqkv[dt // 4, :, (dt % 4) * 1024:(dt % 4 + 1) * 1024])
            for dt in (0, 1, 3, 5):
                w_dt(nc.gpsimd, dt)
            for nm, prm in (("qc", qcos), ("qs", qsin), ("kc", kcos), ("ks", ksin)):
                nc.gpsimd.dma_start(out=cs_tiles[nm][:, 0:256], in_=prm[:, 0:256])
            for dt in (7, 9, 11, 13):
                w_dt(nc.gpsimd, dt)
            for dt in (2, 4, 6, 8, 10, 12, 14):
                w_dt(nc.scalar, dt)
            for dt in (15,):
                w_dt(nc.sync, dt)
            mask_sb = constp.tile([128, 128], BF16, tag="mask")
            nc.scalar.dma_start(out=mask_sb, in_=maskp[:])
            wo_sb = constp.tile([128, HLOC * D], BF16, tag="wo")

            qT = constp.tile([128, HLOC, S], BF16, tag="qT")
            kT = constp.tile([128, KVLOC, S], BF16, tag="kT")
            v_sb = constp.tile([128, ST, KVLOC, VW], BF16, tag="v")
            nc.gpsimd.memset(v_sb, 0.0)
            nc.gpsimd.memset(v_sb[:, :, :, 128:129], 1.0)
            # exp'd score strips, one per local head, reused across pairs
            pTall = constp.tile([128, HLOC, 2 * S], BF16, tag="pT")

            def phase1(i, xs):
                """qkv proj + rmsnorm + rope + transpose for s-tile i"""
                hoff = (i % 2) * 128
                pq = psA.tile([128, 1024], F32, tag="pq")
                for dt in range(DT):
                    lhsT = xs[:, dt * 256 + hoff:dt * 256 + hoff + 128]
                    st, sp = dt == 0, dt == DT - 1
                    nc.tensor.matmul(pq[:, 0:512], lhsT,
                                     w_sb[:, dt * 1024:dt * 1024 + 512],
                                     start=st, stop=sp)
                    nc.tensor.matmul(pq[:, 512:1024], lhsT,
                                     w_sb[:, dt * 1024 + 512:dt * 1024 + 1024],
                                     start=st, stop=sp)

                # rms scales for 6 sub-heads (4 q + 2 k): one wide Square
                # [scalar] + one reduce [vector] instead of 6 accum pairs
                ssq = smallp.tile([128, 8], F32, tag="ssq")
                sqs = workp.tile([128, 768], F32, tag="sqs")
                nc.scalar.activation(sqs, pq[:, 0:768], AF.Square)
                nc.vector.tensor_reduce(ssq[:, 0:6],
                                        sqs.rearrange("p (h d) -> p h d", h=6),
                                        axis=mybir.AxisListType.X, op=ALU.add)
                tm = smallp.tile([128, 8], F32, tag="tm")
                nc.scalar.activation(tm[:, 0:6], ssq[:, 0:6], AF.Sqrt, scale=1.0 / HD)
                scl = smallp.tile([128, 8], F32, tag="scl")
                nc.vector.reciprocal(scl[:, 0:6], tm[:, 0:6])

                qh = workp.tile([128, 6, 128], BF16, tag="qh")
                nc.vector.tensor_mul(qh[:, 0:4], pq[:, 0:512].rearrange("p (h d) -> p h d", h=4),
                                     scl[:, 0:4].unsqueeze(2).broadcast_to([128, 4, 128]))
                nc.vector.tensor_mul(qh[:, 4:6], pq[:, 512:768].rearrange("p (h d) -> p h d", h=2),
                                     scl[:, 4:6].unsqueeze(2).broadcast_to([128, 2, 128]))
                # v: psum -> sbuf bf16
                nc.vector.tensor_copy(v_sb[:, i, :, 0:128],
                                      pq[:, 768:1024].rearrange("p (kv hd) -> p kv hd", kv=2))

                # rope: rp = qh*cosW + swap(qh)*sinW (signs/norm-w folded into
                # tables); broadcast muls on DVE, plain adds on gpsimd
                t1 = workp.tile([128, 6, 128], BF16, tag="t1")
                t2 = workp.tile([128, 6, 128], BF16, tag="t2")
                rp = workp.tile([128, 6, 128], BF16, tag="rp")
                for lo, hi, pre in ((0, 4, "q"), (4, 6, "k")):
                    n = hi - lo
                    cosT = cs_tiles[pre + "c"][:, i * 128:(i + 1) * 128].unsqueeze(1)
                    sinT = cs_tiles[pre + "s"][:, i * 128:(i + 1) * 128].unsqueeze(1)
                    nc.vector.tensor_mul(t1[:, lo:hi], qh[:, lo:hi],
                                         cosT.broadcast_to([128, n, 128]))
                    nc.vector.tensor_mul(t2[:, lo:hi, 0:64], qh[:, lo:hi, 64:128],
                                         sinT[:, :, 0:64].broadcast_to([128, n, 64]))
                    nc.vector.tensor_mul(t2[:, lo:hi, 64:128], qh[:, lo:hi, 0:64],
                                         sinT[:, :, 64:128].broadcast_to([128, n, 64]))
                    nc.vector.tensor_add(rp[:, lo:hi], t1[:, lo:hi], t2[:, lo:hi])

                return rp

            def phase1_trans(i, rp):
                # transpose q/k tiles into [hd, s] layout (one psum tile, 2 copies)
                ptt = psO.tile([128, 768], BF16, tag="po")
                for c in range(6):
                    nc.tensor.transpose(ptt[:, c * 128:(c + 1) * 128], rp[:, c], id_sb)
                nc.vector.tensor_copy(qT[:, :, i * 128:(i + 1) * 128],
                                      ptt[:, 0:512].rearrange("p (h d) -> p h d", h=4))
                nc.vector.tensor_copy(kT[:, :, i * 128:(i + 1) * 128],
                                      ptt[:, 512:768].rearrange("p (h d) -> p h d", h=2))

            aT_a = atp.tile([128, 2, HLOC, 128], BF16, tag="aT")
            aT_b = atp.tile([128, 2, HLOC, 128], BF16, tag="aT")
            aT_tiles = [aT_a, aT_b]

            def attn_qk(m):
                """QK + exp for all 4 heads (scalar runway), then masks."""
                i0, i1 = 2 * m, 2 * m + 1
                for h in range(HLOC):
                    kv = h // 2
                    for g in range(m + 1):
                        ps = psS.tile([128, 512], F32, tag="ps")
                        for jj in range(2):
                            j = 2 * g + jj
                            nc.tensor.matmul(ps[:, jj * 256:(jj + 1) * 256],
                                             kT[:, kv, j * 128:(j + 1) * 128],
                                             qT[:, h, i0 * 128:i0 * 128 + 256],
                                             start=True, stop=True)
                        nc.scalar.activation(pTall[:, h, g * 512:(g + 1) * 512], ps,
                                             AF.Exp, scale=SCALE)
                    nc.vector.tensor_mul(pTall[:, h, i0 * 256:i0 * 256 + 128],
                                         pTall[:, h, i0 * 256:i0 * 256 + 128], mask_sb)
                    nc.vector.tensor_mul(pTall[:, h, i1 * 256 + 128:i1 * 256 + 256],
                                         pTall[:, h, i1 * 256 + 128:i1 * 256 + 256], mask_sb)

            def attn_pv(m):
                """PV + per-head softmax normalize + attn-out transpose."""
                i0 = 2 * m
                aTt = aT_tiles[PAIR_ORDER.index(m) % 2]
                for h in range(HLOC):
                    kv = h // 2
                    ob = workp.tile([128, 2, 128], BF16, tag="ob")
                    rcp = smallp.tile([128, 2], F32, tag="rcp")
                    for t in range(2):
                        i = i0 + t
                        po = psO.tile([128, 132], F32, tag="po")
                        for j in range(i + 1):
                            nc.tensor.matmul(po,
                                             pTall[:, h, j * 256 + t * 128:j * 256 + t * 128 + 128],
                                             v_sb[:, j, kv, :],
                                             start=(j == 0), stop=(j == i))
                        nc.vector.reciprocal(rcp[:, t:t + 1], po[:, 128:129])
                        nc.vector.tensor_mul(ob[:, t], po[:, 0:128],
                                             rcp[:, t:t + 1].broadcast_to([128, 128]))
                    ptt = psO.tile([128, 768], BF16, tag="po")
                    for t in range(2):
                        nc.tensor.transpose(ptt[:, t * 128:(t + 1) * 128], ob[:, t], id_sb)
                    nc.vector.tensor_copy(aTt[:, :, h, :],
                                          ptt[:, 0:256].rearrange("p (t d) -> p t d", t=2))

            def oproj(pair, slot):
                """o_proj partials for both s-tiles of pair; ct-outer
                accumulation so head ct's matmuls start as soon as its
                attn-out transpose lands."""
                aTt = aT_tiles[slot]
                ppa0 = psA.tile([128, 1024], F32, tag="pq")
                ppb0 = psA.tile([128, 1024], F32, tag="pq")
                ppa1 = psA.tile([128, 1024], F32, tag="pq")
                ppb1 = psA.tile([128, 1024], F32, tag="pq")
                pps = [(ppa0, ppb0), (ppa1, ppb1)]
                for ct in range(HLOC):
                    st, sp = ct == 0, ct == HLOC - 1
                    woc = ct * D
                    for t in range(2):
                        ppa, ppb = pps[t]
                        nc.tensor.matmul(ppa[:, 0:512], aTt[:, t, ct, :],
                                         wo_sb[:, woc:woc + 512], start=st, stop=sp)
                        nc.tensor.matmul(ppa[:, 512:1024], aTt[:, t, ct, :],
                                         wo_sb[:, woc + 512:woc + 1024], start=st, stop=sp)
                        nc.tensor.matmul(ppb[:, 0:512], aTt[:, t, ct, :],
                                         wo_sb[:, woc + 1024:woc + 1536], start=st, stop=sp)
                        nc.tensor.matmul(ppb[:, 512:1024], aTt[:, t, ct, :],
                                         wo_sb[:, woc + 1536:woc + 2048], start=st, stop=sp)
                for t in range(2):
                    i = 2 * pair + t
                    ppa, ppb = pps[t]
                    ob2 = workp.tile([128, D], BF16, tag="ob2")
                    nc.vector.tensor_copy(ob2[:, 0:1024], ppa)
                    nc.sync.dma_start(out=out[i * 128:(i + 1) * 128, 0:1024],
                                      in_=ob2[:, 0:1024])
                    nc.vector.tensor_copy(ob2[:, 1024:2048], ppb)
                    nc.sync.dma_start(out=out[i * 128:(i + 1) * 128, 1024:2048],
                                      in_=ob2[:, 1024:2048])

            # ---- phase 1 (pair 0's QK+exp primed mid-phase so phase 2
            # can open with PV(0) immediately) ----
            for m in range(ST // 2):
                if m == 0:
                    xs = xs0
                elif m == 1:
                    xs = xs1
                else:
                    xs = xsp.tile([128, DT * 256], BF16, tag="xs")
                    nc.sync.dma_start(out=xs, in_=xt[m])
                phase1_trans(2 * m, phase1(2 * m, xs))
                if m == 0:
                    nc.scalar.dma_start(out=wo_sb, in_=wot[:])
                    for nm, prm in (("qc", qcos), ("qs", qsin), ("kc", kcos), ("ks", ksin)):
                        nc.gpsimd.dma_start(out=cs_tiles[nm][:, 256:ST * HD],
                                            in_=prm[:, 256:ST * HD])
                rp_last = phase1(2 * m + 1, xs)
                if m == ST // 2 - 1:
                    # fill the rope(15) latency with phase-2 opener PE work
                    attn_pv(0)
                    attn_qk(1)
                phase1_trans(2 * m + 1, rp_last)
                if m == 2:
                    attn_qk(0)

            # ---- phase 2 (PV(0) and QK(1) were emitted in the transition
            # filler): oproj(0), then the QK/oproj/PV pipeline ----
            oproj(0, 0)
            attn_pv(1)
            prev = 1
            for m in PAIR_ORDER[2:]:
                attn_qk(m)
                oproj(prev, PAIR_ORDER.index(prev) % 2)
                attn_pv(m)
                prev = m
            oproj(prev, PAIR_ORDER.index(prev) % 2)
    nc.finalize()
    return nc


def _host_prep(hidden_states, Wq, Wk, Wv, Wo, q_norm_w, k_norm_w):
    bf = ml_dtypes.bfloat16
    inv_freq = 1.0 / (10000.0 ** (np.arange(0, HD, 2, dtype=np.float64) / HD))
    pos = np.arange(S, dtype=np.float64)
    freqs = np.outer(pos, inv_freq)                       # [S, 64]
    emb = np.concatenate([freqs, freqs], axis=-1)         # [S, 128]
    cos = np.cos(emb).astype(np.float32)
    sin = np.sin(emb).astype(np.float32)

    def fold(w):
        # [128, ST*HD]: col st*128+c = table[st*128+row? no: per partition row
        # p (s position within tile), col = s-tile*HD + hd
        w = np.asarray(w, np.float32)
        cosw = (cos * w[None, :]).astype(bf)
        swapsign = np.concatenate([-w[64:], w[:64]])
        sinw = (sin * swapsign[None, :]).astype(bf)
        # [S, HD] -> [ST, 128, HD] -> [128, ST, HD] -> [128, ST*HD]
        cosw = cosw.reshape(ST, 128, HD).transpose(1, 0, 2).reshape(128, ST * HD)
        sinw = sinw.reshape(ST, 128, HD).transpose(1, 0, 2).reshape(128, ST * HD)
        return np.ascontiguousarray(cosw), np.ascontiguousarray(sinw)

    qc, qs = fold(q_norm_w)
    kc, ks = fold(k_norm_w)

    mask = np.triu(np.ones((128, 128), np.float32)).astype(bf)   # [k,q] keep q>=k
    ident = np.eye(128, dtype=np.float32).astype(bf)

    in_maps = []
    for core in range(8):
        b, sh = core // NSH, core % NSH
        xT = np.ascontiguousarray(hidden_states[b].T).astype(bf)     # [D, S]
        # [D, S] -> [DT, 128, ST//2, 256] -> [ST//2, 128(part), DT*256]
        xt = np.ascontiguousarray(
            xT.reshape(DT, 128, ST // 2, 256).transpose(2, 1, 0, 3).reshape(
                ST // 2, 128, DT * 256))
        wq = Wq[sh * OC:(sh + 1) * OC]                                # [512, D]
        wk = Wk[sh * KVLOC * HD:(sh + 1) * KVLOC * HD]                # [256, D]
        wv = Wv[sh * KVLOC * HD:(sh + 1) * KVLOC * HD]
        wcat = np.concatenate([wq, wk, wv], axis=0)                   # [1024, D]
        # W^T [D, 1024] -> [DT, 128, 1024] -> [4, 128, 4*1024] (chunk, part, dt*1024)
        wqkv = wcat.T.astype(bf).reshape(4, 4, 128, 1024).transpose(0, 2, 1, 3)
        wqkv = np.ascontiguousarray(wqkv.reshape(4, 128, 4 * 1024))
        # Wo cols for this shard: [D, 512] -> [HLOC, 128, D] -> [128, HLOC*D]
        wotn = Wo[:, sh * OC:(sh + 1) * OC].T.astype(bf).reshape(HLOC, 128, D)
        wotn = np.ascontiguousarray(wotn.transpose(1, 0, 2).reshape(128, HLOC * D))
        in_maps.append({
            "xt": xt, "wqkv": wqkv, "wot": wotn,
            "qcos": qc, "qsin": qs, "kcos": kc, "ksin": ks,
            "mask": mask, "ident": ident,
        })
    return in_maps


def run(in_maps, **kw):
    if "nc" not in _cache:
        _cache["nc"] = build_nc()
    return run_bass_kernel_spmd(_cache["nc"], in_maps, core_ids=list(range(8)), **kw)


def kernel(**inputs):
    in_maps = _host_prep(**inputs)
    res = run(in_maps).results
    out = np.zeros((B, S, D), np.float32)
    for core in range(8):
        out[core // NSH] += res[core]["out"]
    return out


# revision 45
# speedup vs baseline: 1.1249x; 1.1249x over previous
"""GQA attention kernel for 8 TRN2 NeuronCores.

Sharding: DP over batch (2) x TP over heads (4 shards): each core gets
4 Q heads + 2 KV heads of one batch. Host pre-transposes/retiles inputs,
device computes QKV proj + QK-RMSNorm + RoPE + causal attention + o_proj
partial; host sums the 4 o_proj partials per batch.

Structure (~300us, vs 318.6us baseline; PE busy ~84%):
  Phase 1 (i = 0..15 s-tiles): QKV proj into PSUM [PE]; per-subhead
    one wide Square [scalar] + reduce [vector] for the rms stats;
    Sqrt [scalar]; reciprocal + norm-mul + RoPE (all on vector -- gpsimd
    ops are 2x slower and poisoned the transpose chain); PE transposes
    into persistent qT/kT. Pair 0's QK+exp is primed mid-phase (m==2), and
    the rope(15) latency at the phase boundary is filled by emitting
    PV(0)+QK(1) between tile 15's projection and its transposes.
  Phase 2 (pairs [2,3,4,7,6,5] after the primed 0,1; the big pairs sit
    where o_proj runway exists): per pair, QK scores for all 4 heads
    first (gives the serial Exp stream a runway) [PE] -> Exp [scalar]
    -> causal diag masks [vector] -> o_proj of the PREVIOUS pair
    (ct-outer accumulation in psA, 2 half-row DMAs per s-tile overlap
    cast/writeback) covers the Exp drain -> PV with ones-column
    denominator + normalize [vector] -> attn-out transpose [PE].
  Scalar runs ONLY Square/Sqrt in phase 1 and ONLY Exp in phase 2
  (plus the one primed pair), so activation-table reloads are rare
  (baseline swapped 19x at 1.3us each).

DMA: all host arrays are laid out so every transfer moves 2-16 KB of
contiguous bytes per partition line (512 B lines ran the queues at a
third of peak). w is split into 16 per-dt stripes spread over the 3 HW
queues (sync/scalar/gpsimd) ordered by first-need time; xs0/xs1 lead
the sync queue; wo and the cos/sin bulk are deferred via trigger
placement inside the loop so the startup-critical window only carries
w + xs + cos/sin(tiles 0-1) + ident.

All matmuls run in bf16 (1 PE cycle/row); accumulation is fp32 in PSUM.
No max-subtraction in softmax: RMSNorm bounds |q.k|/sqrt(hd) <= 11.32
so exp() is fp32-safe.
"""

import numpy as np
import ml_dtypes

import concourse.bass as bass
import concourse.mybir as mybir
from concourse import bacc
from concourse.tile import TileContext
from concourse.bass_utils import run_bass_kernel_spmd

B, S, D = 2, 2048, 2048
H, KVH, HD = 16, 8, 128
NSH = 4          # TP shards per batch
HLOC = H // NSH  # 4 q heads per core
KVLOC = KVH // NSH
OC = HLOC * HD   # 512 attn-out channels per core
ST = S // 128    # 16 s-tiles
DT = D // 128    # 16 d-tiles
VW = 132         # v row width: 128 hd + 1 ones + 3 pad
SCALE = 1.0 / np.sqrt(HD)
PAIR_ORDER = [0, 1, 2, 3, 5, 7, 6, 4]

BF16 = mybir.dt.bfloat16
F32 = mybir.dt.float32
AF = mybir.ActivationFunctionType
ALU = mybir.AluOpType

_cache = {}


def build_nc():
    nc = bacc.Bacc()

    xt = nc.declare_dram_parameter("xt", [ST // 2, 128, DT * 256], BF16, isOutput=False)
    wqkv = nc.declare_dram_parameter("wqkv", [4, 128, 4 * 1024], BF16, isOutput=False)
    wot = nc.declare_dram_parameter("wot", [128, HLOC * D], BF16, isOutput=False)
    qcos = nc.declare_dram_parameter("qcos", [128, ST * HD], BF16, isOutput=False)
    qsin = nc.declare_dram_parameter("qsin", [128, ST * HD], BF16, isOutput=False)
    kcos = nc.declare_dram_parameter("kcos", [128, ST * HD], BF16, isOutput=False)
    ksin = nc.declare_dram_parameter("ksin", [128, ST * HD], BF16, isOutput=False)
    maskp = nc.declare_dram_parameter("mask", [128, 128], BF16, isOutput=False)
    identp = nc.declare_dram_parameter("ident", [128, 128], BF16, isOutput=False)
    out = nc.declare_dram_parameter("out", [S, D], BF16, isOutput=True)

    with TileContext(nc) as tc:
        with (
            tc.tile_pool(name="const", bufs=1) as constp,
            tc.tile_pool(name="xs", bufs=2) as xsp,
            tc.tile_pool(name="work", bufs=3) as workp,
            tc.tile_pool(name="small", bufs=4) as smallp,
            tc.tile_pool(name="at", bufs=2) as atp,
            tc.tile_pool(name="psA", bufs=2, space="PSUM") as psA,
            tc.tile_pool(name="psS", bufs=2, space="PSUM") as psS,
            tc.tile_pool(name="psO", bufs=2, space="PSUM") as psO,
        ):
            # ---- persistent tiles / DMA issue order (by first-need time) ----
            # w is split into 16 per-dt transfers round-robined over the 3
            # queues so the dt-loop of tile 0 starts after ~256KB instead of
            # waiting for a whole 1MB chunk. wo and the cos/sin bulk are
            # deferred into the loop, clear of the startup-critical window.
            w_sb = constp.tile([128, DT * 1024], BF16, tag="w")
            xs0 = xsp.tile([128, DT * 256], BF16, tag="xs")
            nc.sync.dma_start(out=xs0, in_=xt[0])
            xs1 = xsp.tile([128, DT * 256], BF16, tag="xs")
            nc.sync.dma_start(out=xs1, in_=xt[1])
            id_sb = constp.tile([128, 128], BF16, tag="ident")
            nc.scalar.dma_start(out=id_sb, in_=identp[:])
            cs_tiles = {}
            for nm, prm in (("qc", qcos), ("qs", qsin), ("kc", kcos), ("ks", ksin)):
                t = constp.tile([128, ST * HD], BF16, tag=nm)
                cs_tiles[nm] = t
            # queue start latencies: gpsimd ~10us, scalar ~12.5, sync ~8.8
            # (but sync first ships xs0). Stripe dts so each arrives just
            # before the dt-loop needs it; xs1 follows on sync.
            def w_dt(eng, dt):
                eng.dma_start(out=w_sb[:, dt * 1024:(dt + 1) * 1024],
                              in_=wqkv[dt // 4, :, (dt % 4) * 1024:(dt % 4 + 1) * 1024])
            for dt in (0, 1, 3, 5):
                w_dt(nc.gpsimd, dt)
            for nm, prm in (("qc", qcos), ("qs", qsin), ("kc", kcos), ("ks", ksin)):
                nc.gpsimd.dma_start(out=cs_tiles[nm][:, 0:256], in_=prm[:, 0:256])
            for dt in (7, 9):
                w_dt(nc.gpsimd, dt)
            for dt in (2, 4, 6, 8, 10):
                w_dt(nc.scalar, dt)
            for dt in (11, 12, 13, 14, 15):
                w_dt(nc.sync, dt)
            mask_sb = constp.tile([128, 128], BF16, tag="mask")
            nc.scalar.dma_start(out=mask_sb, in_=maskp[:])
            wo_sb = constp.tile([128, HLOC * D], BF16, tag="wo")

            qT = constp.tile([128, HLOC, S], BF16, tag="qT")
            kT = constp.tile([128, KVLOC, S], BF16, tag="kT")
            v_sb = constp.tile([128, ST, KVLOC, VW], BF16, tag="v")
            nc.gpsimd.memset(v_sb, 0.0)
            nc.gpsimd.memset(v_sb[:, :, :, 128:129], 1.0)
            # exp'd score strips, one per local head, reused across pairs
            pTall = constp.tile([128, HLOC, 2 * S], BF16, tag="pT")

            def phase1(i, xs):
                """qkv proj + rmsnorm + rope + transpose for s-tile i"""
                hoff = (i % 2) * 128
                pq = psA.tile([128, 1024], F32, tag="pq")
                for dt in range(DT):
                    lhsT = xs[:, dt * 256 + hoff:dt * 256 + hoff + 128]
                    st, sp = dt == 0, dt == DT - 1
                    nc.tensor.matmul(pq[:, 0:512], lhsT,
                                     w_sb[:, dt * 1024:dt * 1024 + 512],
                                     start=st, stop=sp)
                    nc.tensor.matmul(pq[:, 512:1024], lhsT,
                                     w_sb[:, dt * 1024 + 512:dt * 1024 + 1024],
                                     start=st, stop=sp)

                # rms scales for 6 sub-heads (4 q + 2 k): one wide Square
                # [scalar] + one reduce [vector] instead of 6 accum pairs
                ssq = smallp.tile([128, 8], F32, tag="ssq")
                sqs = workp.tile([128, 768], F32, tag="sqs")
                nc.scalar.activation(sqs, pq[:, 0:768], AF.Square)
                nc.vector.tensor_reduce(ssq[:, 0:6],
                                        sqs.rearrange("p (h d) -> p h d", h=6),
                                        axis=mybir.AxisListType.X, op=ALU.add)
                tm = smallp.tile([128, 8], F32, tag="tm")
                nc.scalar.activation(tm[:, 0:6], ssq[:, 0:6], AF.Sqrt, scale=1.0 / HD)
                scl = smallp.tile([128, 8], F32, tag="scl")
                nc.vector.reciprocal(scl[:, 0:6], tm[:, 0:6])

                qh = workp.tile([128, 6, 128], BF16, tag="qh")
                nc.vector.tensor_mul(qh[:, 0:4], pq[:, 0:512].rearrange("p (h d) -> p h d", h=4),
                                     scl[:, 0:4].unsqueeze(2).broadcast_to([128, 4, 128]))
                nc.vector.tensor_mul(qh[:, 4:6], pq[:, 512:768].rearrange("p (h d) -> p h d", h=2),
                                     scl[:, 4:6].unsqueeze(2).broadcast_to([128, 2, 128]))
                # v: psum -> sbuf bf16
                nc.vector.tensor_copy(v_sb[:, i, :, 0:128],
                                      pq[:, 768:1024].rearrange("p (kv hd) -> p kv hd", kv=2))

                # rope: rp = qh*cosW + swap(qh)*sinW (signs/norm-w folded into
                # tables); broadcast muls on DVE, plain adds on gpsimd
                t1 = workp.tile([128, 6, 128], BF16, tag="t1")
                t2 = workp.tile([128, 6, 128], BF16, tag="t2")
                rp = workp.tile([128, 6, 128], BF16, tag="rp")
                for lo, hi, pre in ((0, 4, "q"), (4, 6, "k")):
                    n = hi - lo
                    cosT = cs_tiles[pre + "c"][:, i * 128:(i + 1) * 128].unsqueeze(1)
                    sinT = cs_tiles[pre + "s"][:, i * 128:(i + 1) * 128].unsqueeze(1)
                    nc.vector.tensor_mul(t1[:, lo:hi], qh[:, lo:hi],
                                         cosT.broadcast_to([128, n, 128]))
                    nc.vector.tensor_mul(t2[:, lo:hi, 0:64], qh[:, lo:hi, 64:128],
                                         sinT[:, :, 0:64].broadcast_to([128, n, 64]))
                    nc.vector.tensor_mul(t2[:, lo:hi, 64:128], qh[:, lo:hi, 0:64],
                                         sinT[:, :, 64:128].broadcast_to([128, n, 64]))
                    nc.vector.tensor_add(rp[:, lo:hi], t1[:, lo:hi], t2[:, lo:hi])

                return rp

            def phase1_trans(i, rp):
                # transpose q/k tiles into [hd, s] layout (one psum tile, 2 copies)
                ptt = psO.tile([128, 768], BF16, tag="po")
                for c in range(6):
                    nc.tensor.transpose(ptt[:, c * 128:(c + 1) * 128], rp[:, c], id_sb)
                nc.vector.tensor_copy(qT[:, :, i * 128:(i + 1) * 128],
                                      ptt[:, 0:512].rearrange("p (h d) -> p h d", h=4))
                nc.vector.tensor_copy(kT[:, :, i * 128:(i + 1) * 128],
                                      ptt[:, 512:768].rearrange("p (h d) -> p h d", h=2))

            aT_a = atp.tile([128, 2, HLOC, 128], BF16, tag="aT")
            aT_b = atp.tile([128, 2, HLOC, 128], BF16, tag="aT")
            aT_tiles = [aT_a, aT_b]

            def attn_qk(m):
                """QK + exp for all 4 heads (scalar runway), then masks."""
                i0, i1 = 2 * m, 2 * m + 1
                for h in range(HLOC):
                    kv = h // 2
                    for g in range(m + 1):
                        ps = psS.tile([128, 512], F32, tag="ps")
                        for jj in range(2):
                            j = 2 * g + jj
                            nc.tensor.matmul(ps[:, jj * 256:(jj + 1) * 256],
                                             kT[:, kv, j * 128:(j + 1) * 128],
                                             qT[:, h, i0 * 128:i0 * 128 + 256],
                                             start=True, stop=True)
                        nc.scalar.activation(pTall[:, h, g * 512:(g + 1) * 512], ps,
                                             AF.Exp, scale=SCALE)
                    nc.vector.tensor_mul(pTall[:, h, i0 * 256:i0 * 256 + 128],
                                         pTall[:, h, i0 * 256:i0 * 256 + 128], mask_sb)
                    nc.vector.tensor_mul(pTall[:, h, i1 * 256 + 128:i1 * 256 + 256],
                                         pTall[:, h, i1 * 256 + 128:i1 * 256 + 256], mask_sb)

            def attn_pv(m):
                """PV + per-head softmax normalize + attn-out transpose."""
                i0 = 2 * m
                aTt = aT_tiles[PAIR_ORDER.index(m) % 2]
                for h in range(HLOC):
                    kv = h // 2
                    ob = workp.tile([128, 2, 128], BF16, tag="ob")
                    rcp = smallp.tile([128, 2], F32, tag="rcp")
                    for t in range(2):
                        i = i0 + t
                        po = psO.tile([128, 132], F32, tag="po")
                        for j in range(i + 1):
                            nc.tensor.matmul(po,
                                             pTall[:, h, j * 256 + t * 128:j * 256 + t * 128 + 128],
                                             v_sb[:, j, kv, :],
                                             start=(j == 0), stop=(j == i))
                        nc.vector.reciprocal(rcp[:, t:t + 1], po[:, 128:129])
                        nc.vector.tensor_mul(ob[:, t], po[:, 0:128],
                                             rcp[:, t:t + 1].broadcast_to([128, 128]))
                    ptt = psO.tile([128, 768], BF16, tag="po")
                    for t in range(2):
                        nc.tensor.transpose(ptt[:, t * 128:(t + 1) * 128], ob[:, t], id_sb)
                    nc.vector.tensor_copy(aTt[:, :, h, :],
                                          ptt[:, 0:256].rearrange("p (t d) -> p t d", t=2))

            def oproj(pair, slot):
                """o_proj partials for both s-tiles of pair; ct-outer
                accumulation so head ct's matmuls start as soon as its
                attn-out transpose lands."""
                aTt = aT_tiles[slot]
                ppa0 = psA.tile([128, 1024], F32, tag="pq")
                ppb0 = psA.tile([128, 1024], F32, tag="pq")
                ppa1 = psA.tile([128, 1024], F32, tag="pq")
                ppb1 = psA.tile([128, 1024], F32, tag="pq")
                pps = [(ppa0, ppb0), (ppa1, ppb1)]
                for ct in range(HLOC):
                    st, sp = ct == 0, ct == HLOC - 1
                    woc = ct * D
                    for t in range(2):
                        ppa, ppb = pps[t]
                        nc.tensor.matmul(ppa[:, 0:512], aTt[:, t, ct, :],
                                         wo_sb[:, woc:woc + 512], start=st, stop=sp)
                        nc.tensor.matmul(ppa[:, 512:1024], aTt[:, t, ct, :],
                                         wo_sb[:, woc + 512:woc + 1024], start=st, stop=sp)
                        nc.tensor.matmul(ppb[:, 0:512], aTt[:, t, ct, :],
                                         wo_sb[:, woc + 1024:woc + 1536], start=st, stop=sp)
                        nc.tensor.matmul(ppb[:, 512:1024], aTt[:, t, ct, :],
                                         wo_sb[:, woc + 1536:woc + 2048], start=st, stop=sp)
                for t in range(2):
                    i = 2 * pair + t
                    ppa, ppb = pps[t]
                    ob2 = workp.tile([128, D], BF16, tag="ob2")
                    nc.vector.tensor_copy(ob2[:, 0:1024], ppa)
                    nc.sync.dma_start(out=out[i * 128:(i + 1) * 128, 0:1024],
                                      in_=ob2[:, 0:1024])
                    nc.vector.tensor_copy(ob2[:, 1024:2048], ppb)
                    nc.sync.dma_start(out=out[i * 128:(i + 1) * 128, 1024:2048],
                                      in_=ob2[:, 1024:2048])

            # ---- phase 1 (pair 0's QK+exp primed mid-phase so phase 2
            # can open with PV(0) immediately) ----
            for m in range(ST // 2):
                if m == 0:
                    xs = xs0
                elif m == 1:
                    xs = xs1
                else:
                    xs = xsp.tile([128, DT * 256], BF16, tag="xs")
                    nc.sync.dma_start(out=xs, in_=xt[m])
                phase1_trans(2 * m, phase1(2 * m, xs))
                if m == 0:
                    nc.scalar.dma_start(out=wo_sb, in_=wot[:])
                    for nm, prm in (("qc", qcos), ("qs", qsin), ("kc", kcos), ("ks", ksin)):
                        nc.gpsimd.dma_start(out=cs_tiles[nm][:, 256:ST * HD],
                                            in_=prm[:, 256:ST * HD])
                rp_last = phase1(2 * m + 1, xs)
                if m == ST // 2 - 1:
                    # fill the rope(15) latency with phase-2 opener PE work
                    attn_pv(0)
                    attn_qk(1)
                phase1_trans(2 * m + 1, rp_last)
                if m == 2:
                    attn_qk(0)

            # ---- phase 2 (PV(0) and QK(1) were emitted in the transition
            # filler): oproj(0), then the QK/oproj/PV pipeline ----
            oproj(0, 0)
            attn_pv(1)
            prev = 1
            for m in PAIR_ORDER[2:]:
                attn_qk(m)
                oproj(prev, PAIR_ORDER.index(prev) % 2)
                attn_pv(m)
                prev = m
            oproj(prev, PAIR_ORDER.index(prev) % 2)
    nc.finalize()
    return nc


def _host_prep(hidden_states, Wq, Wk, Wv, Wo, q_norm_w, k_norm_w):
    bf = ml_dtypes.bfloat16
    inv_freq = 1.0 / (10000.0 ** (np.arange(0, HD, 2, dtype=np.float64) / HD))
    pos = np.arange(S, dtype=np.float64)
    freqs = np.outer(pos, inv_freq)                       # [S, 64]
    emb = np.concatenate([freqs, freqs], axis=-1)         # [S, 128]
    cos = np.cos(emb).astype(np.float32)
    sin = np.sin(emb).astype(np.float32)

    def fold(w):
        # [128, ST*HD]: col st*128+c = table[st*128+row? no: per partition row
        # p (s position within tile), col = s-tile*HD + hd
        w = np.asarray(w, np.float32)
        cosw = (cos * w[None, :]).astype(bf)
        swapsign = np.concatenate([-w[64:], w[:64]])
        sinw = (sin * swapsign[None, :]).astype(bf)
        # [S, HD] -> [ST, 128, HD] -> [128, ST, HD] -> [128, ST*HD]
        cosw = cosw.reshape(ST, 128, HD).transpose(1, 0, 2).reshape(128, ST * HD)
        sinw = sinw.reshape(ST, 128, HD).transpose(1, 0, 2).reshape(128, ST * HD)
        return np.ascontiguousarray(cosw), np.ascontiguousarray(sinw)

    qc, qs = fold(q_norm_w)
    kc, ks = fold(k_norm_w)

    mask = np.triu(np.ones((128, 128), np.float32)).astype(bf)   # [k,q] keep q>=k
    ident = np.eye(128, dtype=np.float32).astype(bf)

    in_maps = []
    for core in range(8):
        b, sh = core // NSH, core % NSH
        xT = np.ascontiguousarray(hidden_states[b].T).astype(bf)     # [D, S]
        # [D, S] -> [DT, 128, ST//2, 256] -> [ST//2, 128(part), DT*256]
        xt = np.ascontiguousarray(
            xT.reshape(DT, 128, ST // 2, 256).transpose(2, 1, 0, 3).reshape(
                ST // 2, 128, DT * 256))
        wq = Wq[sh * OC:(sh + 1) * OC]                                # [512, D]
        wk = Wk[sh * KVLOC * HD:(sh + 1) * KVLOC * HD]                # [256, D]
        wv = Wv[sh * KVLOC * HD:(sh + 1) * KVLOC * HD]
        wcat = np.concatenate([wq, wk, wv], axis=0)                   # [1024, D]
        # W^T [D, 1024] -> [DT, 128, 1024] -> [4, 128, 4*1024] (chunk, part, dt*1024)
        wqkv = wcat.T.astype(bf).reshape(4, 4, 128, 1024).transpose(0, 2, 1, 3)
        wqkv = np.ascontiguousarray(wqkv.reshape(4, 128, 4 * 1024))
        # Wo cols for this shard: [D, 512] -> [HLOC, 128, D] -> [128, HLOC*D]
        wotn = Wo[:, sh * OC:(sh + 1) * OC].T.astype(bf).reshape(HLOC, 128, D)
        wotn = np.ascontiguousarray(wotn.transpose(1, 0, 2).reshape(128, HLOC * D))
        in_maps.append({
            "xt": xt, "wqkv": wqkv, "wot": wotn,
            "qcos": qc, "qsin": qs, "kcos": kc, "ksin": ks,
            "mask": mask, "ident": ident,
        })
    return in_maps


def run(in_maps, **kw):
    if "nc" not in _cache:
        _cache["nc"] = build_nc()
    return run_bass_kernel_spmd(_cache["nc"], in_maps, core_ids=list(range(8)), **kw)


def kernel(**inputs):
    in_maps = _host_prep(**inputs)
    res = run(in_maps).results
    out = np.zeros((B, S, D), np.float32)
    for core in range(8):
        out[core // NSH] += res[core]["out"]
    return out


# revision 46
# speedup vs baseline: 1.1310x; 1.0054x over previous
"""GQA attention kernel for 8 TRN2 NeuronCores.

Sharding: DP over batch (2) x TP over heads (4 shards): each core gets
4 Q heads + 2 KV heads of one batch. Host pre-transposes/retiles inputs,
device computes QKV proj + QK-RMSNorm + RoPE + causal attention + o_proj
partial; host sums the 4 o_proj partials per batch.

Structure (~300us, vs 318.6us baseline; PE busy ~84%):
  Phase 1 (i = 0..15 s-tiles): QKV proj into PSUM [PE]; per-subhead
    one wide Square [scalar] + reduce [vector] for the rms stats;
    Sqrt [scalar]; reciprocal + norm-mul + RoPE (all on vector -- gpsimd
    ops are 2x slower and poisoned the transpose chain); PE transposes
    into persistent qT/kT. Pair 0's QK+exp is primed mid-phase (m==2), and
    the rope(15) latency at the phase boundary is filled by emitting
    PV(0)+QK(1) between tile 15's projection and its transposes.
  Phase 2 (pairs [2,3,4,7,6,5] after the primed 0,1; the big pairs sit
    where o_proj runway exists): per pair, QK scores for all 4 heads
    first (gives the serial Exp stream a runway) [PE] -> Exp [scalar]
    -> causal diag masks [vector] -> o_proj of the PREVIOUS pair
    (ct-outer accumulation in psA, 2 half-row DMAs per s-tile overlap
    cast/writeback) covers the Exp drain -> PV with ones-column
    denominator + normalize [vector] -> attn-out transpose [PE].
  Scalar runs ONLY Square/Sqrt in phase 1 and ONLY Exp in phase 2
  (plus the one primed pair), so activation-table reloads are rare
  (baseline swapped 19x at 1.3us each).

DMA: all host arrays are laid out so every transfer moves 2-16 KB of
contiguous bytes per partition line (512 B lines ran the queues at a
third of peak). w is split into 16 per-dt stripes spread over the 3 HW
queues (sync/scalar/gpsimd) ordered by first-need time; xs0/xs1 lead
the sync queue; wo and the cos/sin bulk are deferred via trigger
placement inside the loop so the startup-critical window only carries
w + xs + cos/sin(tiles 0-1) + ident.

All matmuls run in bf16 (1 PE cycle/row); accumulation is fp32 in PSUM.
No max-subtraction in softmax: RMSNorm bounds |q.k|/sqrt(hd) <= 11.32
so exp() is fp32-safe.
"""

import numpy as np
import ml_dtypes

import concourse.bass as bass
import concourse.mybir as mybir
from concourse import bacc
from concourse.tile import TileContext
from concourse.bass_utils import run_bass_kernel_spmd

B, S, D = 2, 2048, 2048
H, KVH, HD = 16, 8, 128
NSH = 4          # TP shards per batch
HLOC = H // NSH  # 4 q heads per core
KVLOC = KVH // NSH
OC = HLOC * HD   # 512 attn-out channels per core
ST = S // 128    # 16 s-tiles
DT = D // 128    # 16 d-tiles
VW = 132         # v row width: 128 hd + 1 ones + 3 pad
SCALE = 1.0 / np.sqrt(HD)
PAIR_ORDER = [0, 1, 2, 3, 4, 7, 6, 5]

BF16 = mybir.dt.bfloat16
F32 = mybir.dt.float32
AF = mybir.ActivationFunctionType
ALU = mybir.AluOpType

_cache = {}


def build_nc():
    nc = bacc.Bacc()

    xt = nc.declare_dram_parameter("xt", [ST // 2, 128, DT * 256], BF16, isOutput=False)
    wqkv = nc.declare_dram_parameter("wqkv", [4, 128, 4 * 1024], BF16, isOutput=False)
    wot = nc.declare_dram_parameter("wot", [128, HLOC * D], BF16, isOutput=False)
    qcos = nc.declare_dram_parameter("qcos", [128, ST * HD], BF16, isOutput=False)
    qsin = nc.declare_dram_parameter("qsin", [128, ST * HD], BF16, isOutput=False)
    kcos = nc.declare_dram_parameter("kcos", [128, ST * HD], BF16, isOutput=False)
    ksin = nc.declare_dram_parameter("ksin", [128, ST * HD], BF16, isOutput=False)
    maskp = nc.declare_dram_parameter("mask", [128, 128], BF16, isOutput=False)
    identp = nc.declare_dram_parameter("ident", [128, 128], BF16, isOutput=False)
    out = nc.declare_dram_parameter("out", [S, D], BF16, isOutput=True)

    with TileContext(nc) as tc:
        with (
            tc.tile_pool(name="const", bufs=1) as constp,
            tc.tile_pool(name="xs", bufs=2) as xsp,
            tc.tile_pool(name="work", bufs=3) as workp,
            tc.tile_pool(name="small", bufs=4) as smallp,
            tc.tile_pool(name="at", bufs=2) as atp,
            tc.tile_pool(name="psA", bufs=2, space="PSUM") as psA,
            tc.tile_pool(name="psS", bufs=2, space="PSUM") as psS,
            tc.tile_pool(name="psO", bufs=2, space="PSUM") as psO,
        ):
            # ---- persistent tiles / DMA issue order (by first-need time) ----
            # w is split into 16 per-dt transfers round-robined over the 3
            # queues so the dt-loop of tile 0 starts after ~256KB instead of
            # waiting for a whole 1MB chunk. wo and the cos/sin bulk are
            # deferred into the loop, clear of the startup-critical window.
            w_sb = constp.tile([128, DT * 1024], BF16, tag="w")
            xs0 = xsp.tile([128, DT * 256], BF16, tag="xs")
            nc.sync.dma_start(out=xs0, in_=xt[0])
            xs1 = xsp.tile([128, DT * 256], BF16, tag="xs")
            nc.sync.dma_start(out=xs1, in_=xt[1])
            id_sb = constp.tile([128, 128], BF16, tag="ident")
            nc.scalar.dma_start(out=id_sb, in_=identp[:])
            cs_tiles = {}
            for nm, prm in (("qc", qcos), ("qs", qsin), ("kc", kcos), ("ks", ksin)):
                t = constp.tile([128, ST * HD], BF16, tag=nm)
                cs_tiles[nm] = t
            # queue start latencies: gpsimd ~10us, scalar ~12.5, sync ~8.8
            # (but sync first ships xs0). Stripe dts so each arrives just
            # before the dt-loop needs it; xs1 follows on sync.
            def w_dt(eng, dt):
                eng.dma_start(out=w_sb[:, dt * 1024:(dt + 1) * 1024],
                              in_=wqkv[dt // 4, :, (dt % 4) * 1024:(dt % 4 + 1) * 1024])
            for dt in (0, 1, 3, 5):
                w_dt(nc.gpsimd, dt)
            for nm, prm in (("qc", qcos), ("qs", qsin), ("kc", kcos), ("ks", ksin)):
                nc.gpsimd.dma_start(out=cs_tiles[nm][:, 0:256], in_=prm[:, 0:256])
            for dt in (7, 9):
                w_dt(nc.gpsimd, dt)
            for dt in (2, 4, 6, 8, 10):
                w_dt(nc.scalar, dt)
            for dt in (11, 12, 13, 14, 15):
                w_dt(nc.sync, dt)
            mask_sb = constp.tile([128, 128], BF16, tag="mask")
            nc.scalar.dma_start(out=mask_sb, in_=maskp[:])
            wo_sb = constp.tile([128, HLOC * D], BF16, tag="wo")

            qT = constp.tile([128, HLOC, S], BF16, tag="qT")
            kT = constp.tile([128, KVLOC, S], BF16, tag="kT")
            v_sb = constp.tile([128, ST, KVLOC, VW], BF16, tag="v")
            nc.gpsimd.memset(v_sb, 0.0)
            nc.gpsimd.memset(v_sb[:, :, :, 128:129], 1.0)
            # exp'd score strips, one per local head, reused across pairs
            pTall = constp.tile([128, HLOC, 2 * S], BF16, tag="pT")

            def phase1(i, xs):
                """qkv proj + rmsnorm + rope + transpose for s-tile i"""
                hoff = (i % 2) * 128
                pq = psA.tile([128, 1024], F32, tag="pq")
                for dt in range(DT):
                    lhsT = xs[:, dt * 256 + hoff:dt * 256 + hoff + 128]
                    st, sp = dt == 0, dt == DT - 1
                    nc.tensor.matmul(pq[:, 0:512], lhsT,
                                     w_sb[:, dt * 1024:dt * 1024 + 512],
                                     start=st, stop=sp)
                    nc.tensor.matmul(pq[:, 512:1024], lhsT,
                                     w_sb[:, dt * 1024 + 512:dt * 1024 + 1024],
                                     start=st, stop=sp)

                # rms scales for 6 sub-heads (4 q + 2 k): one wide Square
                # [scalar] + one reduce [vector] instead of 6 accum pairs
                ssq = smallp.tile([128, 8], F32, tag="ssq")
                sqs = workp.tile([128, 768], F32, tag="sqs")
                nc.scalar.activation(sqs, pq[:, 0:768], AF.Square)
                nc.vector.tensor_reduce(ssq[:, 0:6],
                                        sqs.rearrange("p (h d) -> p h d", h=6),
                                        axis=mybir.AxisListType.X, op=ALU.add)
                tm = smallp.tile([128, 8], F32, tag="tm")
                nc.scalar.activation(tm[:, 0:6], ssq[:, 0:6], AF.Sqrt, scale=1.0 / HD)
                scl = smallp.tile([128, 8], F32, tag="scl")
                nc.vector.reciprocal(scl[:, 0:6], tm[:, 0:6])

                qh = workp.tile([128, 6, 128], BF16, tag="qh")
                nc.vector.tensor_mul(qh[:, 0:4], pq[:, 0:512].rearrange("p (h d) -> p h d", h=4),
                                     scl[:, 0:4].unsqueeze(2).broadcast_to([128, 4, 128]))
                nc.vector.tensor_mul(qh[:, 4:6], pq[:, 512:768].rearrange("p (h d) -> p h d", h=2),
                                     scl[:, 4:6].unsqueeze(2).broadcast_to([128, 2, 128]))
                # v: psum -> sbuf bf16
                nc.vector.tensor_copy(v_sb[:, i, :, 0:128],
                                      pq[:, 768:1024].rearrange("p (kv hd) -> p kv hd", kv=2))

                # rope: rp = qh*cosW + swap(qh)*sinW (signs/norm-w folded into
                # tables); broadcast muls on DVE, plain adds on gpsimd
                t1 = workp.tile([128, 6, 128], BF16, tag="t1")
                t2 = workp.tile([128, 6, 128], BF16, tag="t2")
                rp = workp.tile([128, 6, 128], BF16, tag="rp")
                for lo, hi, pre in ((0, 4, "q"), (4, 6, "k")):
                    n = hi - lo
                    cosT = cs_tiles[pre + "c"][:, i * 128:(i + 1) * 128].unsqueeze(1)
                    sinT = cs_tiles[pre + "s"][:, i * 128:(i + 1) * 128].unsqueeze(1)
                    nc.vector.tensor_mul(t1[:, lo:hi], qh[:, lo:hi],
                                         cosT.broadcast_to([128, n, 128]))
                    nc.vector.tensor_mul(t2[:, lo:hi, 0:64], qh[:, lo:hi, 64:128],
                                         sinT[:, :, 0:64].broadcast_to([128, n, 64]))
                    nc.vector.tensor_mul(t2[:, lo:hi, 64:128], qh[:, lo:hi, 0:64],
                                         sinT[:, :, 64:128].broadcast_to([128, n, 64]))
                    nc.vector.tensor_add(rp[:, lo:hi], t1[:, lo:hi], t2[:, lo:hi])

                return rp

            def phase1_trans(i, rp):
                # transpose q/k tiles into [hd, s] layout (one psum tile, 2 copies)
                ptt = psO.tile([128, 768], BF16, tag="po")
                for c in range(6):
                    nc.tensor.transpose(ptt[:, c * 128:(c + 1) * 128], rp[:, c], id_sb)
                nc.vector.tensor_copy(qT[:, :, i * 128:(i + 1) * 128],
                                      ptt[:, 0:512].rearrange("p (h d) -> p h d", h=4))
                nc.vector.tensor_copy(kT[:, :, i * 128:(i + 1) * 128],
                                      ptt[:, 512:768].rearrange("p (h d) -> p h d", h=2))

            aT_a = atp.tile([128, 2, HLOC, 128], BF16, tag="aT")
            aT_b = atp.tile([128, 2, HLOC, 128], BF16, tag="aT")
            aT_tiles = [aT_a, aT_b]

            def attn_qk(m):
                """QK + exp for all 4 heads (scalar runway), then masks."""
                i0, i1 = 2 * m, 2 * m + 1
                for h in range(HLOC):
                    kv = h // 2
                    for g in range(m + 1):
                        ps = psS.tile([128, 512], F32, tag="ps")
                        for jj in range(2):
                            j = 2 * g + jj
                            nc.tensor.matmul(ps[:, jj * 256:(jj + 1) * 256],
                                             kT[:, kv, j * 128:(j + 1) * 128],
                                             qT[:, h, i0 * 128:i0 * 128 + 256],
                                             start=True, stop=True)
                        nc.scalar.activation(pTall[:, h, g * 512:(g + 1) * 512], ps,
                                             AF.Exp, scale=SCALE)
                    nc.vector.tensor_mul(pTall[:, h, i0 * 256:i0 * 256 + 128],
                                         pTall[:, h, i0 * 256:i0 * 256 + 128], mask_sb)
                    nc.vector.tensor_mul(pTall[:, h, i1 * 256 + 128:i1 * 256 + 256],
                                         pTall[:, h, i1 * 256 + 128:i1 * 256 + 256], mask_sb)

            def attn_pv(m):
                """PV + per-head softmax normalize + attn-out transpose."""
                i0 = 2 * m
                aTt = aT_tiles[PAIR_ORDER.index(m) % 2]
                for h in range(HLOC):
                    kv = h // 2
                    ob = workp.tile([128, 2, 128], BF16, tag="ob")
                    rcp = smallp.tile([128, 2], F32, tag="rcp")
                    for t in range(2):
                        i = i0 + t
                        po = psO.tile([128, 132], F32, tag="po")
                        for j in range(i + 1):
                            nc.tensor.matmul(po,
                                             pTall[:, h, j * 256 + t * 128:j * 256 + t * 128 + 128],
                                             v_sb[:, j, kv, :],
                                             start=(j == 0), stop=(j == i))
                        nc.vector.reciprocal(rcp[:, t:t + 1], po[:, 128:129])
                        nc.vector.tensor_mul(ob[:, t], po[:, 0:128],
                                             rcp[:, t:t + 1].broadcast_to([128, 128]))
                    ptt = psO.tile([128, 768], BF16, tag="po")
                    for t in range(2):
                        nc.tensor.transpose(ptt[:, t * 128:(t + 1) * 128], ob[:, t], id_sb)
                    nc.vector.tensor_copy(aTt[:, :, h, :],
                                          ptt[:, 0:256].rearrange("p (t d) -> p t d", t=2))

            def oproj(pair, slot):
                """o_proj partials for both s-tiles of pair; ct-outer
                accumulation so head ct's matmuls start as soon as its
                attn-out transpose lands."""
                aTt = aT_tiles[slot]
                ppa0 = psA.tile([128, 1024], F32, tag="pq")
                ppb0 = psA.tile([128, 1024], F32, tag="pq")
                ppa1 = psA.tile([128, 1024], F32, tag="pq")
                ppb1 = psA.tile([128, 1024], F32, tag="pq")
                pps = [(ppa0, ppb0), (ppa1, ppb1)]
                for ct in range(HLOC):
                    st, sp = ct == 0, ct == HLOC - 1
                    woc = ct * D
                    for t in range(2):
                        ppa, ppb = pps[t]
                        nc.tensor.matmul(ppa[:, 0:512], aTt[:, t, ct, :],
                                         wo_sb[:, woc:woc + 512], start=st, stop=sp)
                        nc.tensor.matmul(ppa[:, 512:1024], aTt[:, t, ct, :],
                                         wo_sb[:, woc + 512:woc + 1024], start=st, stop=sp)
                        nc.tensor.matmul(ppb[:, 0:512], aTt[:, t, ct, :],
                                         wo_sb[:, woc + 1024:woc + 1536], start=st, stop=sp)
                        nc.tensor.matmul(ppb[:, 512:1024], aTt[:, t, ct, :],
                                         wo_sb[:, woc + 1536:woc + 2048], start=st, stop=sp)
                for t in range(2):
                    i = 2 * pair + t
                    ppa, ppb = pps[t]
                    ob2 = workp.tile([128, D], BF16, tag="ob2")
                    nc.vector.tensor_copy(ob2[:, 0:1024], ppa)
                    nc.sync.dma_start(out=out[i * 128:(i + 1) * 128, 0:1024],
                                      in_=ob2[:, 0:1024])
                    nc.vector.tensor_copy(ob2[:, 1024:2048], ppb)
                    nc.sync.dma_start(out=out[i * 128:(i + 1) * 128, 1024:2048],
                                      in_=ob2[:, 1024:2048])

            # ---- phase 1 (pair 0's QK+exp primed mid-phase so phase 2
            # can open with PV(0) immediately) ----
            for m in range(ST // 2):
                if m == 0:
                    xs = xs0
                elif m == 1:
                    xs = xs1
                else:
                    xs = xsp.tile([128, DT * 256], BF16, tag="xs")
                    nc.sync.dma_start(out=xs, in_=xt[m])
                phase1_trans(2 * m, phase1(2 * m, xs))
                if m == 0:
                    nc.scalar.dma_start(out=wo_sb, in_=wot[:])
                    for nm, prm in (("qc", qcos), ("qs", qsin), ("kc", kcos), ("ks", ksin)):
                        nc.gpsimd.dma_start(out=cs_tiles[nm][:, 256:ST * HD],
                                            in_=prm[:, 256:ST * HD])
                rp_last = phase1(2 * m + 1, xs)
                if m == ST // 2 - 1:
                    # fill the rope(15) latency with phase-2 opener PE work
                    attn_pv(0)
                    attn_qk(1)
                phase1_trans(2 * m + 1, rp_last)
                if m == 2:
                    attn_qk(0)

            # ---- phase 2 (PV(0) and QK(1) were emitted in the transition
            # filler): oproj(0), then the QK/oproj/PV pipeline ----
            oproj(0, 0)
            attn_pv(1)
            prev = 1
            for m in PAIR_ORDER[2:]:
                attn_qk(m)
                oproj(prev, PAIR_ORDER.index(prev) % 2)
                attn_pv(m)
                prev = m
            oproj(prev, PAIR_ORDER.index(prev) % 2)
    nc.finalize()
    return nc


def _host_prep(hidden_states, Wq, Wk, Wv, Wo, q_norm_w, k_norm_w):
    bf = ml_dtypes.bfloat16
    inv_freq = 1.0 / (10000.0 ** (np.arange(0, HD, 2, dtype=np.float64) / HD))
    pos = np.arange(S, dtype=np.float64)
    freqs = np.outer(pos, inv_freq)                       # [S, 64]
    emb = np.concatenate([freqs, freqs], axis=-1)         # [S, 128]
    cos = np.cos(emb).astype(np.float32)
    sin = np.sin(emb).astype(np.float32)

    def fold(w):
        # [128, ST*HD]: col st*128+c = table[st*128+row? no: per partition row
        # p (s position within tile), col = s-tile*HD + hd
        w = np.asarray(w, np.float32)
        cosw = (cos * w[None, :]).astype(bf)
        swapsign = np.concatenate([-w[64:], w[:64]])
        sinw = (sin * swapsign[None, :]).astype(bf)
        # [S, HD] -> [ST, 128, HD] -> [128, ST, HD] -> [128, ST*HD]
        cosw = cosw.reshape(ST, 128, HD).transpose(1, 0, 2).reshape(128, ST * HD)
        sinw = sinw.reshape(ST, 128, HD).transpose(1, 0, 2).reshape(128, ST * HD)
        return np.ascontiguousarray(cosw), np.ascontiguousarray(sinw)

    qc, qs = fold(q_norm_w)
    kc, ks = fold(k_norm_w)

    mask = np.triu(np.ones((128, 128), np.float32)).astype(bf)   # [k,q] keep q>=k
    ident = np.eye(128, dtype=np.float32).astype(bf)

    in_maps = []
    for core in range(8):
        b, sh = core // NSH, core % NSH
        xT = np.ascontiguousarray(hidden_states[b].T).astype(bf)     # [D, S]
        # [D, S] -> [DT, 128, ST//2, 256] -> [ST//2, 128(part), DT*256]
        xt = np.ascontiguousarray(
            xT.reshape(DT, 128, ST // 2, 256).transpose(2, 1, 0, 3).reshape(
                ST // 2, 128, DT * 256))
        wq = Wq[sh * OC:(sh + 1) * OC]                                # [512, D]
        wk = Wk[sh * KVLOC * HD:(sh + 1) * KVLOC * HD]                # [256, D]
        wv = Wv[sh * KVLOC * HD:(sh + 1) * KVLOC * HD]
        wcat = np.concatenate([wq, wk, wv], axis=0)                   # [1024, D]
        # W^T [D, 1024] -> [DT, 128, 1024] -> [4, 128, 4*1024] (chunk, part, dt*1024)
        wqkv = wcat.T.astype(bf).reshape(4, 4, 128, 1024).transpose(0, 2, 1, 3)
        wqkv = np.ascontiguousarray(wqkv.reshape(4, 128, 4 * 1024))
        # Wo cols for this shard: [D, 512] -> [HLOC, 128, D] -> [128, HLOC*D]
        wotn = Wo[:, sh * OC:(sh + 1) * OC].T.astype(bf).reshape(HLOC, 128, D)
        wotn = np.ascontiguousarray(wotn.transpose(1, 0, 2).reshape(128, HLOC * D))
        in_maps.append({
            "xt": xt, "wqkv": wqkv, "wot": wotn,
            "qcos": qc, "qsin": qs, "kcos": kc, "ksin": ks,
            "mask": mask, "ident": ident,
        })
    return in_maps


def run(in_maps, **kw):
    if "nc" not in _cache:
        _cache["nc"] = build_nc()
    return run_bass_kernel_spmd(_cache["nc"], in_maps, core_ids=list(range(8)), **kw)


def kernel(**inputs):
    in_maps = _host_prep(**inputs)
    res = run(in_maps).results
    out = np.zeros((B, S, D), np.float32)
    for core in range(8):
        out[core // NSH] += res[core]["out"]
    return out
